# revision 4
# baseline (speedup 1.0000x reference)
"""Trainium2 Bass kernel for nn_DotMatrix.

Math: for each (b, ell, t) the reference computes a complex pairwise dot
matrix O[i,j] = sum_m z[i,m] * w[j,m] where z = rep[b,:,t,:,:] as complex
and w is the sign-flipped conjugation partner.  As a real matmul:

  lhsT[k, i]   k = (c,m) stacked: [Zr.T; Zi.T]                 [2m, 256]
  rhs[k, 2j+c'] c'=0: [FZr; -FZi], c'=1: [FZi; FZr]            [2m, 512]
  out = lhsT.T @ rhs  -> [256 i, 512 (j,c)]

with FZr[m',j] = s[m'] * Zr[j, M-1-m'], s[m'] = (-1)^(ell+m').

Sharding: 8 cores = 2 batches x 4 tau-quarters.  Each core owns 32
channels ch = ell*8 + s (t = tq*8 + s) and computes the full 256x256
pairwise matrix for each -> per-core output [32, 2, 128, 512] written as
dense contiguous 256KB blocks; the host reassembles the full
[2, 256, 256, 128, 2] tensor.

Device kernel per core: 64 matmuls (ch x i-half) of [K=16, M=128] x
[K=16, N=512] fp32.  Channels are packed 4-per-partition-group (base
partitions 0/32/64/96) so input DMAs span all 128 partitions.
"""

import numpy as np

import concourse.bass as bass
import concourse.bacc as bacc
import concourse.mybir as mybir
from concourse.bass_utils import run_bass_kernel_spmd
from concourse.tile import TileContext

B, N, TAU, NELL = 2, 256, 32, 4
NCORES = 8
NCH = 32          # channels per core (4 ell * 8 t)
NSLOT = 8         # channel slots per partition group
F32 = mybir.dt.float32

_NC_CACHE = {}


def _build_bass():
    nc = bacc.Bacc()
    lhs = nc.declare_dram_parameter("lhs", [128, NSLOT * 256], F32, isOutput=False)
    rhs = nc.declare_dram_parameter("rhs", [128, NSLOT * 512], F32, isOutput=False)
    out = nc.declare_dram_parameter("out", [NCH, 2, 128, 512], F32, isOutput=True)

    with TileContext(nc) as tc:
        with (
            tc.tile_pool(name="lin", bufs=1) as lin_pool,
            tc.tile_pool(name="rin", bufs=1) as rin_pool,
            tc.tile_pool(name="ps", bufs=8, space="PSUM") as ps_pool,
            tc.tile_pool(name="ot", bufs=8) as ot_pool,
        ):
            lhs_sb = lin_pool.tile([128, NSLOT * 256], F32)
            rhs_sb = rin_pool.tile([128, NSLOT * 512], F32)
            # chunked input loads (2 slots each) so compute starts early
            for q in range(4):
                nc.gpsimd.dma_start(
                    out=lhs_sb[:, q * 512 : (q + 1) * 512],
                    in_=lhs[:, q * 512 : (q + 1) * 512],
                )
                nc.gpsimd.dma_start(
                    out=rhs_sb[:, q * 1024 : (q + 1) * 1024],
                    in_=rhs[:, q * 1024 : (q + 1) * 1024],
                )
            n_copy = 0
            for slot in range(NSLOT):
                for g in range(4):
                    ch = slot * 4 + g
                    for half in range(2):
                        ps = ps_pool.tile([128, 512], F32)
                        nc.tensor.matmul(
                            ps[:],
                            lhs_sb[
                                32 * g : 32 * g + 16,
                                slot * 256 + half * 128 : slot * 256 + (half + 1) * 128,
                            ],
                            rhs_sb[32 * g : 32 * g + 16, slot * 512 : (slot + 1) * 512],
                            start=True,
                            stop=True,
                            tile_position=(32 * g, 0),
                        )
                        ot = ot_pool.tile([128, 512], F32)
                        if n_copy % 2 == 0:
                            nc.scalar.copy(ot[:], ps[:])
                        else:
                            nc.vector.tensor_copy(out=ot[:], in_=ps[:])
                        n_copy += 1
                        nc.sync.dma_start(out=out[ch, half], in_=ot[:])
    nc.compile()
    return nc


def _host_prep(reps, cid):
    """Build per-core lhs/rhs host tensors."""
    b, tq = cid // 4, cid % 4
    LHS = np.zeros((128, NSLOT * 256), np.float32)
    RHS = np.zeros((128, NSLOT * 512), np.float32)
    for ell in range(NELL):
        rep = reps[ell]
        m = 2 * ell + 1
        s_vec = ((-1.0) ** (ell + np.arange(m))).astype(np.float32)
        for sidx in range(8):
            t = tq * 8 + sidx
            ch = ell * 8 + sidx
            g, slot = ch % 4, ch // 4
            Z = rep[b, :, t]                      # [256, m, 2]
            Zr, Zi = Z[..., 0], Z[..., 1]         # [256, m]
            lhsT = np.concatenate([Zr.T, Zi.T], axis=0)      # [2m, 256]
            FZr = s_vec[:, None] * Zr[:, ::-1].T             # [m, 256]
            FZi = s_vec[:, None] * Zi[:, ::-1].T
            R = np.empty((2 * m, 256, 2), np.float32)
            R[0:m, :, 0] = FZr
            R[m :, :, 0] = -FZi
            R[0:m, :, 1] = FZi
            R[m :, :, 1] = FZr
            LHS[32 * g : 32 * g + 2 * m, slot * 256 : (slot + 1) * 256] = lhsT
            RHS[32 * g : 32 * g + 2 * m, slot * 512 : (slot + 1) * 512] = R.reshape(
                2 * m, 512
            )
    return {"lhs": LHS, "rhs": RHS}


def _run(in_maps, **kw):
    if "nc" not in _NC_CACHE:
        _NC_CACHE["nc"] = _build_bass()
    return run_bass_kernel_spmd(_NC_CACHE["nc"], in_maps, list(range(NCORES)), **kw)


def kernel(rep0, rep1, rep2, rep3, _bass_kw=None):
    reps = [np.ascontiguousarray(np.asarray(r, dtype=np.float32)) for r in (rep0, rep1, rep2, rep3)]
    in_maps = [_host_prep(reps, cid) for cid in range(NCORES)]
    res = _run(in_maps, **(_bass_kw or {}))
    out = np.empty((B, N, N, NELL * TAU, 2), np.float32)
    for cid in range(NCORES):
        b, tq = cid // 4, cid % 4
        o = res.results[cid]["out"].reshape(NCH, 256, 256, 2)
        for ell in range(NELL):
            lo = ell * TAU + tq * 8
            out[b, :, :, lo : lo + 8, :] = o[ell * 8 : (ell + 1) * 8].transpose(
                1, 2, 0, 3
            )
    kernel.last_result = res
    return out


# revision 7
# speedup vs baseline: 1.0949x; 1.0949x over previous
"""Trainium2 Bass kernel for nn_DotMatrix.

Math: for each (b, ell, t) the reference computes a complex pairwise dot
matrix O[i,j] = sum_m z[i,m] * w[j,m] where z = rep[b,:,t,:,:] as complex
and w the sign-flipped conjugation partner.  As a real matmul:

  lhsT[k, i]   k = (c,m) stacked: [Zr.T; Zi.T]                 [2m, 256]
  rhs[k, 2j+c'] c'=0: [FZr; -FZi], c'=1: [FZi; FZr]            [2m, 512]
  out = lhsT.T @ rhs  -> [256 i, 512 (j,c)]

with FZr[m',j] = s[m'] * Zr[j, M-1-m'], s[m'] = (-1)^(ell+m').

Precision trick: fp32 matmuls run at 4 cycles/column on the PE, but the
contraction dim here is tiny (2m <= 14), so we decompose each operand
into three bf16 parts (hi/mid/lo, 24 mantissa bits total) and stack the
six significant cross terms along the dead K dimension:

  L = [Ah; Am; Al; Ah; Am; Ah]   R = [Bh; Bh; Bh; Bm; Bm; Bl]

One bf16 matmul (K = 6*2m <= 84) then equals the fp32 product to
~2^-24, at 1 cycle/column — 4x faster than the fp32 path and with fast
(FWL) weight loads.

Sharding: 8 cores = 2 batches x 4 tau-quarters.  Each core owns 32
channels ch = ell*8 + s (t = tq*8 + s), computes the full 256x256
matrix per channel, and writes [32, 2, 128, 512] fp32; channel pairs
share one 1MB contiguous DMA.  Host reassembles [2,256,256,128,2].
"""

import numpy as np
import ml_dtypes

import concourse.bass as bass
import concourse.bacc as bacc
import concourse.mybir as mybir
from concourse.bass_utils import run_bass_kernel_spmd
from concourse.tile import TileContext

B, N, TAU, NELL = 2, 256, 32, 4
NCORES = 8
NCH = 32          # channels per core (4 ell * 8 slots)
F32 = mybir.dt.float32
BF16 = mybir.dt.bfloat16
BFNP = ml_dtypes.bfloat16
KS = [6 * 2 * (2 * ell + 1) for ell in range(NELL)]   # 12, 36, 60, 84

_NC_CACHE = {}


def _build_bass():
    nc = bacc.Bacc()
    lhs_d = [
        nc.declare_dram_parameter(f"lhs{e}", [KS[e], 8 * 256], BF16, isOutput=False)
        for e in range(NELL)
    ]
    rhs_d = [
        nc.declare_dram_parameter(f"rhs{e}", [KS[e], 8 * 512], BF16, isOutput=False)
        for e in range(NELL)
    ]
    out = nc.declare_dram_parameter("out", [NCH, 2, 128, 512], F32, isOutput=True)

    with TileContext(nc) as tc:
        with (
            tc.tile_pool(name="lin", bufs=1) as lin_pool,
            tc.tile_pool(name="rin", bufs=1) as rin_pool,
            tc.tile_pool(name="ps", bufs=8, space="PSUM") as ps_pool,
            tc.tile_pool(name="ot", bufs=5) as ot_pool,
        ):
            lhs_sb = [lin_pool.tile([KS[e], 8 * 256], BF16, tag=f"l{e}", name=f"lhs_sb{e}") for e in range(NELL)]
            rhs_sb = [rin_pool.tile([KS[e], 8 * 512], BF16, tag=f"r{e}", name=f"rhs_sb{e}") for e in range(NELL)]
            # chunked input loads (4 slots each) so compute starts early
            for e in range(NELL):
                for c in range(2):
                    nc.gpsimd.dma_start(
                        out=lhs_sb[e][:, c * 1024 : (c + 1) * 1024],
                        in_=lhs_d[e][:, c * 1024 : (c + 1) * 1024],
                    )
                    nc.gpsimd.dma_start(
                        out=rhs_sb[e][:, c * 2048 : (c + 1) * 2048],
                        in_=rhs_d[e][:, c * 2048 : (c + 1) * 2048],
                    )
            n_copy = 0
            for e in range(NELL):
                K = KS[e]
                for u in range(4):              # channel pair within ell
                    ot = ot_pool.tile([128, 2048], F32)
                    for c2 in range(2):         # channel within pair
                        sl = u * 2 + c2
                        for half in range(2):
                            ps = ps_pool.tile([128, 512], F32)
                            nc.tensor.matmul(
                                ps[:],
                                lhs_sb[e][
                                    0:K,
                                    sl * 256 + half * 128 : sl * 256 + (half + 1) * 128,
                                ],
                                rhs_sb[e][0:K, sl * 512 : (sl + 1) * 512],
                                start=True,
                                stop=True,
                            )
                            dst = ot[:, c2 * 1024 + half * 512 : c2 * 1024 + (half + 1) * 512]
                            if n_copy % 2 == 0:
                                nc.scalar.copy(dst, ps[:])
                            else:
                                nc.vector.tensor_copy(out=dst, in_=ps[:])
                            n_copy += 1
                    pair = e * 8 + u * 2
                    nc.sync.dma_start(
                        out=out[pair : pair + 2].rearrange("c h p x -> p c h x"),
                        in_=ot[:].rearrange("p (c h x) -> p c h x", c=2, h=2),
                    )
    nc.compile()
    return nc


def _dec3(x):
    h = x.astype(BFNP)
    r = x - h.astype(np.float32)
    m_ = r.astype(BFNP)
    l = (r - m_.astype(np.float32)).astype(BFNP)
    return h, m_, l


def _host_prep(reps, cid):
    """Build per-core bf16 K-stacked lhs/rhs tensors."""
    b, tq = cid // 4, cid % 4
    im = {}
    for ell in range(NELL):
        rep = reps[ell]
        m = 2 * ell + 1
        s_vec = ((-1.0) ** (ell + np.arange(m))).astype(np.float32)
        LHS = np.empty((KS[ell], 8 * 256), BFNP)
        RHS = np.empty((KS[ell], 8 * 512), BFNP)
        for sidx in range(8):
            t = tq * 8 + sidx
            Z = rep[b, :, t]                      # [256, m, 2]
            Zr, Zi = Z[..., 0], Z[..., 1]         # [256, m]
            lhsT = np.concatenate([Zr.T, Zi.T], axis=0)      # [2m, 256]
            FZr = s_vec[:, None] * Zr[:, ::-1].T             # [m, 256]
            FZi = s_vec[:, None] * Zi[:, ::-1].T
            R = np.empty((2 * m, 256, 2), np.float32)
            R[0:m, :, 0] = FZr
            R[m:, :, 0] = -FZi
            R[0:m, :, 1] = FZi
            R[m:, :, 1] = FZr
            rhs = R.reshape(2 * m, 512)
            Ah, Am, Al = _dec3(lhsT)
            Bh, Bm, Bl = _dec3(rhs)
            LHS[:, sidx * 256 : (sidx + 1) * 256] = np.concatenate(
                [Ah, Am, Al, Ah, Am, Ah], axis=0
            )
            RHS[:, sidx * 512 : (sidx + 1) * 512] = np.concatenate(
                [Bh, Bh, Bh, Bm, Bm, Bl], axis=0
            )
        im[f"lhs{ell}"] = LHS
        im[f"rhs{ell}"] = RHS
    return im


def _run(in_maps, **kw):
    if "nc" not in _NC_CACHE:
        _NC_CACHE["nc"] = _build_bass()
    return run_bass_kernel_spmd(_NC_CACHE["nc"], in_maps, list(range(NCORES)), **kw)


def kernel(rep0, rep1, rep2, rep3, _bass_kw=None):
    reps = [np.ascontiguousarray(np.asarray(r, dtype=np.float32)) for r in (rep0, rep1, rep2, rep3)]
    in_maps = [_host_prep(reps, cid) for cid in range(NCORES)]
    res = _run(in_maps, **(_bass_kw or {}))
    out = np.empty((B, N, N, NELL * TAU, 2), np.float32)
    for cid in range(NCORES):
        b, tq = cid // 4, cid % 4
        o = res.results[cid]["out"].reshape(NCH, 256, 256, 2)
        for ell in range(NELL):
            lo = ell * TAU + tq * 8
            out[b, :, :, lo : lo + 8, :] = o[ell * 8 : (ell + 1) * 8].transpose(
                1, 2, 0, 3
            )
    kernel.last_result = res
    return out


# revision 11
# speedup vs baseline: 1.1079x; 1.0118x over previous
"""Trainium2 Bass kernel for nn_DotMatrix.

Math: for each (b, ell, t) the reference computes a complex pairwise dot
matrix O[i,j] = sum_m z[i,m] * w[j,m] where z = rep[b,:,t,:,:] as complex
and w the sign-flipped conjugation partner.  As a real matmul:

  lhsT[k, i]   k = (c,m) stacked: [Zr.T; Zi.T]                 [2m, 256]
  rhs[k, 2j+c'] c'=0: [FZr; -FZi], c'=1: [FZi; FZr]            [2m, 512]
  out = lhsT.T @ rhs  -> [256 i, 512 (j,c)]

with FZr[m',j] = s[m'] * Zr[j, M-1-m'], s[m'] = (-1)^(ell+m').

Precision trick: fp32 matmuls run at 4 cycles/column on the PE, but the
contraction dim here is tiny (2m <= 14), so we decompose each operand
into three bf16 parts (hi/mid/lo, 24 mantissa bits total) and stack the
six significant cross terms along the dead K dimension:

  L = [Ah; Am; Al; Ah; Am; Ah]   R = [Bh; Bh; Bh; Bm; Bm; Bl]

One bf16 matmul (K = 6*2m <= 84) then equals the fp32 product to
~2^-24, at 1 cycle/column — 4x faster than the fp32 path and with fast
(FWL) weight loads.

Sharding: 8 cores = 2 batches x 4 tau-quarters.  Each core owns 32
channels ch = ell*8 + s (t = tq*8 + s), computes the full 256x256
matrix per channel, and writes [32, 2, 128, 512] fp32; channel pairs
share one 1MB contiguous DMA.  Host reassembles [2,256,256,128,2].
"""

import numpy as np
import ml_dtypes

import concourse.bass as bass
import concourse.bacc as bacc
import concourse.mybir as mybir
from concourse.bass_utils import run_bass_kernel_spmd
from concourse.tile import TileContext

B, N, TAU, NELL = 2, 256, 32, 4
NCORES = 8
NCH = 32          # channels per core (4 ell * 8 slots)
F32 = mybir.dt.float32
BF16 = mybir.dt.bfloat16
BFNP = ml_dtypes.bfloat16
KS = [6 * 2 * (2 * ell + 1) for ell in range(NELL)]   # 12, 36, 60, 84

_NC_CACHE = {}


def _build_bass():
    nc = bacc.Bacc()
    lhs_d = [
        nc.declare_dram_parameter(f"lhs{e}", [KS[e], 8 * 256], BF16, isOutput=False)
        for e in range(NELL)
    ]
    rhs_d = [
        nc.declare_dram_parameter(f"rhs{e}", [KS[e], 8 * 512], BF16, isOutput=False)
        for e in range(NELL)
    ]
    # [pair, i%128, (ch%2, i//128, jc)] — matches the SBUF out tile exactly
    # so each pair is one fully-contiguous 1MB DMA; host un-permutes.
    out = nc.declare_dram_parameter("out", [NCH // 2, 128, 2048], F32, isOutput=True)

    with TileContext(nc) as tc:
        with (
            tc.tile_pool(name="lin", bufs=1) as lin_pool,
            tc.tile_pool(name="rin", bufs=1) as rin_pool,
            tc.tile_pool(name="ps", bufs=8, space="PSUM") as ps_pool,
            tc.tile_pool(name="ot", bufs=5) as ot_pool,
        ):
            lhs_sb = [lin_pool.tile([KS[e], 8 * 256], BF16, tag=f"l{e}", name=f"lhs_sb{e}") for e in range(NELL)]
            rhs_sb = [rin_pool.tile([KS[e], 8 * 512], BF16, tag=f"r{e}", name=f"rhs_sb{e}") for e in range(NELL)]
            # input loads via HWDGE; ell-major order matches compute order
            for e in range(NELL):
                nc.scalar.dma_start(out=lhs_sb[e][:], in_=lhs_d[e][:])
                nc.scalar.dma_start(out=rhs_sb[e][:], in_=rhs_d[e][:])
            n_copy = 0
            for e in range(NELL):
                K = KS[e]
                for u in range(4):              # channel pair within ell
                    ot = ot_pool.tile([128, 2048], F32)
                    for c2 in range(2):         # channel within pair
                        sl = u * 2 + c2
                        for half in range(2):
                            ps = ps_pool.tile([128, 512], F32)
                            nc.tensor.matmul(
                                ps[:],
                                lhs_sb[e][
                                    0:K,
                                    sl * 256 + half * 128 : sl * 256 + (half + 1) * 128,
                                ],
                                rhs_sb[e][0:K, sl * 512 : (sl + 1) * 512],
                                start=True,
                                stop=True,
                            )
                            dst = ot[:, c2 * 1024 + half * 512 : c2 * 1024 + (half + 1) * 512]
                            if n_copy % 2 == 0:
                                nc.scalar.copy(dst, ps[:])
                            else:
                                nc.vector.tensor_copy(out=dst, in_=ps[:])
                            n_copy += 1
                    nc.sync.dma_start(out=out[e * 4 + u], in_=ot[:])
    nc.compile()
    return nc


def _dec3(x):
    h = x.astype(BFNP)
    r = x - h.astype(np.float32)
    m_ = r.astype(BFNP)
    l = (r - m_.astype(np.float32)).astype(BFNP)
    return h, m_, l


def _host_prep(reps, cid):
    """Build per-core bf16 K-stacked lhs/rhs tensors."""
    b, tq = cid // 4, cid % 4
    im = {}
    for ell in range(NELL):
        rep = reps[ell]
        m = 2 * ell + 1
        s_vec = ((-1.0) ** (ell + np.arange(m))).astype(np.float32)
        LHS = np.empty((KS[ell], 8 * 256), BFNP)
        RHS = np.empty((KS[ell], 8 * 512), BFNP)
        for sidx in range(8):
            t = tq * 8 + sidx
            Z = rep[b, :, t]                      # [256, m, 2]
            Zr, Zi = Z[..., 0], Z[..., 1]         # [256, m]
            lhsT = np.concatenate([Zr.T, Zi.T], axis=0)      # [2m, 256]
            FZr = s_vec[:, None] * Zr[:, ::-1].T             # [m, 256]
            FZi = s_vec[:, None] * Zi[:, ::-1].T
            R = np.empty((2 * m, 256, 2), np.float32)
            R[0:m, :, 0] = FZr
            R[m:, :, 0] = -FZi
            R[0:m, :, 1] = FZi
            R[m:, :, 1] = FZr
            rhs = R.reshape(2 * m, 512)
            Ah, Am, Al = _dec3(lhsT)
            Bh, Bm, Bl = _dec3(rhs)
            LHS[:, sidx * 256 : (sidx + 1) * 256] = np.concatenate(
                [Ah, Am, Al, Ah, Am, Ah], axis=0
            )
            RHS[:, sidx * 512 : (sidx + 1) * 512] = np.concatenate(
                [Bh, Bh, Bh, Bm, Bm, Bl], axis=0
            )
        im[f"lhs{ell}"] = LHS
        im[f"rhs{ell}"] = RHS
    return im


def _run(in_maps, **kw):
    if "nc" not in _NC_CACHE:
        _NC_CACHE["nc"] = _build_bass()
    return run_bass_kernel_spmd(_NC_CACHE["nc"], in_maps, list(range(NCORES)), **kw)


def kernel(rep0, rep1, rep2, rep3, _bass_kw=None):
    reps = [np.ascontiguousarray(np.asarray(r, dtype=np.float32)) for r in (rep0, rep1, rep2, rep3)]
    in_maps = [_host_prep(reps, cid) for cid in range(NCORES)]
    res = _run(in_maps, **(_bass_kw or {}))
    out = np.empty((B, N, N, NELL * TAU, 2), np.float32)
    for cid in range(NCORES):
        b, tq = cid // 4, cid % 4
        o = (
            res.results[cid]["out"]
            .reshape(NCH // 2, 128, 2, 2, 512)
            .transpose(0, 2, 3, 1, 4)
            .reshape(NCH, 256, 256, 2)
        )
        for ell in range(NELL):
            lo = ell * TAU + tq * 8
            out[b, :, :, lo : lo + 8, :] = o[ell * 8 : (ell + 1) * 8].transpose(
                1, 2, 0, 3
            )
    kernel.last_result = res
    return out
